# revision 3
# baseline (speedup 1.0000x reference)
"""Builder for the LSTM-encoder (VAE head) Trainium kernel.

Distribution: hidden-split across 8 cores. Each core computes gates for its
128 hidden units (512 gate columns, order [g|i|f|o]) for the FULL batch
B=128, batch-major [B, 512] in PSUM. The hidden state h is exchanged each
step via an 8-core AllGather of the core's transposed chunk hT_k [128, B].

Math per step (core k, gate column slice S_k):
    gates = x_t @ W[:, S_k] + h @ U[:, S_k] + b[S_k]        # [B, 512]
    g = softplus(gates[:, 0:128]); i,f,o = sigmoid(gates[:, 128:512])
    c = f*c + i*g ; h_k = o * softplus(c)                   # [B, 128]
    hT_k = h_k.T  --AllGather-->  hT [1024, B] for step t+1

Head (all cores redundantly): muT = Wm.T @ hT + bm, logvarT likewise,
zT = muT + epsT * exp(0.5*logvarT). Host transposes back.

mm_dt: dtype of all TensorE-facing tensors (float32r = full-rate fp32
matmul for moving free dim >= 256; bfloat16 also possible).
"""

import numpy as np

import concourse.bass as bass
import concourse.mybir as mybir
import concourse.tile as tile
from concourse.masks import make_identity
from concourse.tile import add_dep_helper

AF = mybir.ActivationFunctionType
F32 = mybir.dt.float32

B, D, H, Z = 128, 256, 1024, 128
NCORES = 8
HS = H // NCORES          # 128 hidden units per core
GC = 4 * HS               # 512 gate columns per core
KH = H // 128             # 8 hT chunks
KD = D // 128             # 2 x chunks


# Engine-compute ISA structs carry a limited number of sync-wait slots
# (matmul: 1 usable on the MM struct; small fixed counts elsewhere). Tile's
# sem assigner can emit more. Spill the excess onto standalone
# EventSemaphore wait instructions inserted just before the offender in the
# same engine stream — identical semantics, a few ns of dispatch.
_SPILL_TYPES = (
    "InstMatmult",
    "InstTensorTensor",
    "InstActivation",
    "InstTensorCopy",
    "InstTensorScalarPtr",
    "InstReciprocal",
    "InstMemset",
    "InstNoOp",
    "InstLdweights",
    "InstCopyPredicated",
    "InstTensorScalarAffineSelect",
    "InstCollectiveCompute",
    "InstEventSemaphore",
    "InstDrain",
    "InstDMACopy",
)


_WAIT_LIMITS = {}


def _spill_excess_waits(nc, limit=1):
    f = nc.m.functions[0]
    n_spilled = 0
    for bb in f.blocks:
        out = []
        for inst in bb.instructions:
            si = inst.sync_info
            waits = list(si.on_wait) if si and si.on_wait else []
            tname = type(inst).__name__
            limit_t = _WAIT_LIMITS.get(tname, limit)
            if tname in _SPILL_TYPES and len(waits) > limit_t:
                keep = waits[len(waits) - limit_t :]
                for w in waits[: len(waits) - limit_t]:
                    es = mybir.InstEventSemaphore(
                        name=f"WSPILL-{n_spilled}-{inst.name}",
                        engine=inst.engine,
                        ins=[],
                        outs=[],
                        sync_info=mybir.SyncInfo(on_wait=[w], on_update=[]),
                    )
                    out.append(es)
                    n_spilled += 1
                si.on_wait = keep
            out.append(inst)
        bb.instructions = out
    return n_spilled


def build_nc(T: int, mm_dt=mybir.dt.bfloat16, use_cc=True, T_data=None, n_dev=NCORES):
    if T_data is None:
        T_data = T
    nc = bass.Bass(
        "TRN2", target_bir_lowering=False, debug=False, num_devices=n_dev
    )

    xT_d = nc.dram_tensor("xT", [T_data, D, B], mm_dt, kind="ExternalInput")
    U_d = nc.dram_tensor("Usl", [KH, 128, GC], mm_dt, kind="ExternalInput")
    W_d = nc.dram_tensor("Wsl", [KD, 128, GC], mm_dt, kind="ExternalInput")
    b_d = nc.dram_tensor("bsl", [1, GC], mm_dt, kind="ExternalInput")
    Wm_d = nc.dram_tensor("Wm", [KH, 128, Z], mm_dt, kind="ExternalInput")
    Wv_d = nc.dram_tensor("Wv", [KH, 128, Z], mm_dt, kind="ExternalInput")
    bm_d = nc.dram_tensor("bm", [Z, 1], F32, kind="ExternalInput")
    bv_d = nc.dram_tensor("bv", [Z, 1], F32, kind="ExternalInput")
    bvh_d = nc.dram_tensor("bvh", [Z, 1], F32, kind="ExternalInput")
    epsT_d = nc.dram_tensor("epsT", [Z, B], F32, kind="ExternalInput")

    muT_d = nc.dram_tensor("muT", [Z, B], F32, kind="ExternalOutput")
    lvT_d = nc.dram_tensor("logvarT", [Z, B], F32, kind="ExternalOutput")
    zT_d = nc.dram_tensor("zT", [Z, B], F32, kind="ExternalOutput")

    with tile.TileContext(nc) as tc:
        with (
            tc.tile_pool(name="const", bufs=1) as cpool,
            tc.tile_pool(name="xt", bufs=6) as xt_pool,
            tc.tile_pool(name="hTg", bufs=3) as hT_pool,
            tc.tile_pool(name="gps", bufs=3, space="PSUM") as gps_pool,
            tc.tile_pool(name="trps", bufs=2, space="PSUM") as trps_pool,
            tc.tile_pool(name="headps", bufs=1, space="PSUM") as head_pool,
            tc.tile_pool(name="act", bufs=3) as apool,
            tc.tile_pool(name="ccd", bufs=2, space="DRAM") as dpool,
        ):
            # ---- constants / persistent state ----
            U_sb = cpool.tile([128, KH, GC], mm_dt, tag="U")
            nc.sync.dma_start(out=U_sb[:], in_=U_d.ap().rearrange("c p g -> p c g"))
            W_sb = cpool.tile([128, KD, GC], mm_dt, tag="W")
            nc.sync.dma_start(out=W_sb[:], in_=W_d.ap().rearrange("c p g -> p c g"))
            b_sb = cpool.tile([1, GC], mm_dt, tag="b")
            nc.sync.dma_start(out=b_sb[:], in_=b_d.ap())
            ones_sb = cpool.tile([1, B], mm_dt, tag="ones")
            nc.vector.memset(ones_sb[:], 1.0)
            ident = cpool.tile([128, 128], mybir.dt.bfloat16, tag="ident")
            make_identity(nc, ident[:])
            c_sb = cpool.tile([128, HS], F32, tag="c")
            nc.vector.memset(c_sb[:], 0.0)

            Wm_sb = cpool.tile([128, KH, Z], mm_dt, tag="Wm")
            nc.sync.dma_start(out=Wm_sb[:], in_=Wm_d.ap().rearrange("c p z -> p c z"))
            Wv_sb = cpool.tile([128, KH, Z], mm_dt, tag="Wv")
            nc.sync.dma_start(out=Wv_sb[:], in_=Wv_d.ap().rearrange("c p z -> p c z"))
            bm_sb = cpool.tile([Z, 1], F32, tag="bm")
            nc.sync.dma_start(out=bm_sb[:], in_=bm_d.ap())
            bv_sb = cpool.tile([Z, 1], F32, tag="bv")
            nc.sync.dma_start(out=bv_sb[:], in_=bv_d.ap())
            bvh_sb = cpool.tile([Z, 1], F32, tag="bvh")
            nc.sync.dma_start(out=bvh_sb[:], in_=bvh_d.ap())
            epsT_sb = cpool.tile([Z, B], F32, tag="epsT")
            nc.sync.dma_start(out=epsT_sb[:], in_=epsT_d.ap())

            # initial gathered hidden state (zeros)
            hT_prev = hT_pool.tile([128, KH, B], mm_dt, tag="hT")
            nc.vector.memset(hT_prev[:], 0.0)

            xT_r = xT_d.ap().rearrange("t (c p) b -> t p c b", p=128)

            # warm-up transpose: folds the identity-matrix (gpsimd) dep into
            # the PE clock so per-step transposes carry a single wait
            tr_warm = trps_pool.tile([128, B], mybir.dt.bfloat16, tag="trps")
            nc.tensor.transpose(tr_warm[:], ident[:], ident[:])

            # psum reader instruction of step t (for the wait-absorbing nop)
            psum_reader = [None, None, None]  # indexed by t % gps bufs

            for t in range(T):
                xt = xt_pool.tile([128, KD, B], mm_dt, tag="xt")
                nc.sync.dma_start(out=xt[:], in_=xT_r[t % T_data])

                # A matmul ISA instruction carries at most ONE output-side
                # sync wait. The start=True matmul of this step's PSUM group
                # needs (a) WAR vs the ACT reader from step t-2 (slot reuse)
                # and (b) the PE drain self-wait. Absorb (a) into a PE nop.
                rd = psum_reader[t % 3]
                if rd is not None:
                    pe_nop = nc.tensor.nop()
                    add_dep_helper(pe_nop.ins, rd.ins, sync=True)

                g_ps = gps_pool.tile([128, GC], F32, tag="gps")
                mm0 = nc.tensor.matmul(
                    g_ps[:], ones_sb[:], b_sb[:], start=True, stop=False
                )
                if rd is not None:
                    add_dep_helper(mm0.ins, pe_nop.ins, sync=True)
                for c in range(KD):
                    nc.tensor.matmul(
                        g_ps[:],
                        xt[:, c, :],
                        W_sb[:, c, :],
                        start=False,
                        stop=False,
                    )
                for c in range(KH):
                    nc.tensor.matmul(
                        g_ps[:],
                        hT_prev[:, c, :],
                        U_sb[:, c, :],
                        start=False,
                        stop=(c == KH - 1),
                    )

                # activations. Host negates the g-block weight columns, so
                # PSUM holds [-g | i | f | o]; one Exp(scale=-1) read gives
                # e_all = [e^g | e^-i | e^-f | e^-o].
                # softplus(x) = Ln(exp(x) + 1); sigmoid(x) = 1/(1+exp(-x))
                # (no HW softplus/sigmoid LUT in the Exp/Ln table set)
                e_all = apool.tile([128, GC], mybir.dt.bfloat16, tag="eall")
                exp_inst = nc.scalar.activation(
                    e_all[:], g_ps[:], AF.Exp, scale=-1.0
                )
                psum_reader[t % 3] = exp_inst
                spg = apool.tile([128, HS], F32, tag="spg")
                nc.scalar.activation(spg[:], e_all[:, 0:HS], AF.Ln, bias=1.0)
                d_ifo = apool.tile([128, 3 * HS], mybir.dt.bfloat16, tag="difo")
                nc.vector.tensor_scalar_add(d_ifo[:], e_all[:, HS:GC], 1.0)
                sig = apool.tile([128, 3 * HS], mybir.dt.bfloat16, tag="sig")
                with nc.allow_low_precision("bf16 sigmoid is fine"):
                    nc.vector.reciprocal(sig[:], d_ifo[:])

                t_ig = apool.tile([128, HS], F32, tag="tig")
                nc.vector.tensor_mul(t_ig[:], sig[:, 0:HS], spg[:])
                t_fc = apool.tile([128, HS], F32, tag="tfc")
                nc.vector.tensor_mul(t_fc[:], sig[:, HS : 2 * HS], c_sb[:])
                nc.vector.tensor_add(c_sb[:], t_ig[:], t_fc[:])

                ec = apool.tile([128, HS], mybir.dt.bfloat16, tag="ec")
                nc.scalar.activation(ec[:], c_sb[:], AF.Exp)
                spc = apool.tile([128, HS], F32, tag="spc")
                nc.scalar.activation(spc[:], ec[:], AF.Ln, bias=1.0)
                h_sb = apool.tile([128, HS], mybir.dt.bfloat16, tag="h")
                nc.vector.tensor_mul(h_sb[:], sig[:, 2 * HS : 3 * HS], spc[:])

                # transpose h -> [hid, B]
                tr_ps = trps_pool.tile([128, B], mybir.dt.bfloat16, tag="trps")
                nc.tensor.transpose(tr_ps[:], h_sb[:], ident[:])
                hT_mine = apool.tile([128, B], mm_dt, tag="hTm")
                nc.vector.tensor_copy(hT_mine[:], tr_ps[:])

                # AllGather hT_k -> hT (all chunks)
                if use_cc == "off":
                    # no cross-step state exchange (timing experiments only)
                    continue
                cc_in = dpool.tile([128, B], mm_dt, tag="ccin")
                nc.sync.dma_start(out=cc_in[:], in_=hT_mine[:])
                cc_out = dpool.tile([KH * 128, B], mm_dt, tag="ccout")
                if use_cc is True:
                    nc.gpsimd.collective_compute(
                        "AllGather",
                        mybir.AluOpType.bypass,
                        replica_groups=[list(range(NCORES))],
                        ins=[cc_in[:].opt()],
                        outs=[cc_out[:].opt()],
                    )
                elif use_cc == "local":
                    for cch in range(KH):
                        nc.sync.dma_start(
                            out=cc_out[:].rearrange("(c p) b -> c p b", p=128)[cch],
                            in_=hT_mine[:],
                        )
                hT_prev = hT_pool.tile([128, KH, B], mm_dt, tag="hT")
                nc.sync.dma_start(
                    out=hT_prev[:],
                    in_=cc_out[:].rearrange("(c p) b -> p c b", p=128),
                )

            # ---- VAE head ----
            mu_ps = head_pool.tile([Z, B], F32, tag="head_mu")
            lv_ps = head_pool.tile([Z, B], F32, tag="head_lv")
            for c in range(KH):
                nc.tensor.matmul(
                    mu_ps[:],
                    Wm_sb[:, c, :],
                    hT_prev[:, c, :],
                    start=(c == 0),
                    stop=(c == KH - 1),
                )
            for c in range(KH):
                nc.tensor.matmul(
                    lv_ps[:],
                    Wv_sb[:, c, :],
                    hT_prev[:, c, :],
                    start=(c == 0),
                    stop=(c == KH - 1),
                )

            mu_sb = apool.tile([Z, B], F32, tag="mu")
            nc.scalar.activation(mu_sb[:], mu_ps[:], AF.Identity, bias=bm_sb[:])
            lv_sb = apool.tile([Z, B], F32, tag="lv")
            nc.scalar.activation(lv_sb[:], lv_ps[:], AF.Identity, bias=bv_sb[:])
            e_sb = apool.tile([Z, B], F32, tag="e")
            nc.scalar.activation(
                e_sb[:], lv_ps[:], AF.Exp, bias=bvh_sb[:], scale=0.5
            )
            ez = apool.tile([Z, B], F32, tag="ez")
            nc.vector.tensor_mul(ez[:], e_sb[:], epsT_sb[:])
            z_sb = apool.tile([Z, B], F32, tag="z")
            nc.vector.tensor_add(z_sb[:], mu_sb[:], ez[:])

            nc.sync.dma_start(out=muT_d.ap(), in_=mu_sb[:])
            nc.sync.dma_start(out=lvT_d.ap(), in_=lv_sb[:])
            nc.sync.dma_start(out=zT_d.ap(), in_=z_sb[:])

    _spill_excess_waits(nc)
    return nc


def make_in_maps(x, W, U, b, Wm, bm, Wv, bv, eps, np_mm_dtype=None):
    import ml_dtypes
    if np_mm_dtype is None:
        np_mm_dtype = ml_dtypes.bfloat16
    """Host-side pre-processing: transpose x, slice/permute weights per core."""
    T = x.shape[1]
    xT = np.ascontiguousarray(x.transpose(1, 2, 0)).astype(np_mm_dtype)  # [T,D,B]
    epsT = np.ascontiguousarray(eps.T).astype(np.float32)  # [Z, B]
    bm_c = np.ascontiguousarray(bm.reshape(Z, 1)).astype(np.float32)
    bv_c = np.ascontiguousarray(bv.reshape(Z, 1)).astype(np.float32)
    bvh_c = np.ascontiguousarray(0.5 * bv.reshape(Z, 1)).astype(np.float32)
    Wm_r = np.ascontiguousarray(Wm.reshape(KH, 128, Z)).astype(np_mm_dtype)
    Wv_r = np.ascontiguousarray(Wv.reshape(KH, 128, Z)).astype(np_mm_dtype)

    in_maps = []
    for k in range(NCORES):
        # gate order [g | i | f | o]; Keras kernel order is i,f,g,o
        cols = np.concatenate(
            [
                np.arange(2 * H + k * HS, 2 * H + (k + 1) * HS),  # g
                np.arange(0 * H + k * HS, 0 * H + (k + 1) * HS),  # i
                np.arange(1 * H + k * HS, 1 * H + (k + 1) * HS),  # f
                np.arange(3 * H + k * HS, 3 * H + (k + 1) * HS),  # o
            ]
        )
        # negate the g-block columns so one Exp(scale=-1) serves all gates
        neg = np.ones((GC,), np.float32)
        neg[:HS] = -1.0
        Usl = np.ascontiguousarray(U[:, cols] * neg).reshape(KH, 128, GC).astype(
            np_mm_dtype
        )
        Wsl = np.ascontiguousarray(W[:, cols] * neg).reshape(KD, 128, GC).astype(
            np_mm_dtype
        )
        bsl = (b[cols] * neg).reshape(1, GC).astype(np_mm_dtype)
        in_maps.append(
            {
                "xT": xT,
                "Usl": Usl,
                "Wsl": Wsl,
                "bsl": bsl,
                "Wm": Wm_r,
                "Wv": Wv_r,
                "bm": bm_c,
                "bv": bv_c,
                "bvh": bvh_c,
                "epsT": epsT,
            }
        )
    return in_maps


def postprocess(core0_out):
    mu = np.ascontiguousarray(core0_out["muT"].T).astype(np.float32)
    logvar = np.ascontiguousarray(core0_out["logvarT"].T).astype(np.float32)
    z = np.ascontiguousarray(core0_out["zT"].T).astype(np.float32)
    return mu, logvar, z


# ----------------------------------------------------------------------------
# Harness entry point: full (unsharded) inputs -> full outputs.
# ----------------------------------------------------------------------------
_NC_CACHE = {}


def kernel(x, W, U, b, Wm, bm, Wv, bv, eps):
    import time as _time

    from concourse.bass_utils import run_bass_kernel_spmd

    x = np.asarray(x, dtype=np.float32)
    W = np.asarray(W, dtype=np.float32)
    U = np.asarray(U, dtype=np.float32)
    b = np.asarray(b, dtype=np.float32)
    Wm = np.asarray(Wm, dtype=np.float32)
    bm = np.asarray(bm, dtype=np.float32)
    Wv = np.asarray(Wv, dtype=np.float32)
    bv = np.asarray(bv, dtype=np.float32)
    eps = np.asarray(eps, dtype=np.float32)

    T = x.shape[1]
    if T not in _NC_CACHE:
        _NC_CACHE[T] = build_nc(T)
    nc = _NC_CACHE[T]
    in_maps = make_in_maps(x, W, U, b, Wm, bm, Wv, bv, eps)
    last = None
    for _attempt in range(3):
        try:
            res = run_bass_kernel_spmd(nc, in_maps, core_ids=list(range(NCORES)))
            return postprocess(res.results[0])
        except Exception as e:  # transient device hiccups: retry
            last = e
            _time.sleep(2.0)
    raise last
